# revision 24
# baseline (speedup 1.0000x reference)
"""Trainium2 Bass kernel for nn_NeuralNet_19250043421419.

Row-normalize x (mean/std over D=3072, ddof=1) then a 3-layer MLP
(3072->32->32->10) with LeakyReLU(0.01) after every layer.

Strategy: pure data parallel over 8 NeuronCores (batch 32768 -> 4096/core).
Per core, per 512-row block:
  - DMA x in natural layout, casting fp32->fp16 in the SWDGE DMA.  GpSimd
    does nothing else, so DMA issue is never gated by compute.
  - Per-row sum(x^2)/(D-1) split between DVE (scalar_tensor_tensor with
    accum) and ACT (Square activation with accum), tunable.
  - PE transposes x into [d, i] tiles with the dedicated transpose datapath
    (fp16 PSUM output: halves both PSUM banks and copy cost); the
    PSUM->SBUF copies are split between DVE (fp16 2x rate) and ACT.
  - PE streams the transposed tiles against [w1^T | 1/D] (M=33),
    accumulating y0_raw in PSUM rows 0..31 and the row-mean in row 32.
  - Normalization folds in afterwards: (x-m)/s @ w1^T =
    (y0_raw - m * rowsum(w1)) / s.  The mean row moves to partition 0 via
    a small SBUF->SBUF HWDGE DMA, the K=1 mean-correction matmul
    accumulates onto the closed PSUM group, and the 1/s scaling is a DVE
    multiply against a stream_shuffle-broadcast row, with
    var = ssq/(D-1) - m^2 * D/(D-1) (Bessel).
  - The post stage for block b-1 is interleaved into block b's emission so
    every engine sees its dependencies ready on arrival (PE post matmuls
    are spread between transpose pairs).
  - PE transposes the [10, 512] result back to natural [512, 10] and DMAs out.
"""
import os
import sys

for _p in ("/opt/trn_rl_repo", "/root/.axon_site/_ro/trn_rl_repo"):
    if os.path.isdir(_p) and _p not in sys.path:
        sys.path.append(_p)

import numpy as np

import concourse.bass as bass
import concourse.bacc as bacc
import concourse.tile as tile
from concourse import mybir
from concourse.bass_utils import run_bass_kernel_spmd

F32 = mybir.dt.float32
F16 = mybir.dt.float16
AF = mybir.ActivationFunctionType
ALU = mybir.AluOpType

N_CORES = 8
B = 32768
D = 3072
H = 32
O = 10
M = 33                     # GEMM stationary cols: [w1^T | 1/D]
MP = 32                    # partition of the mean row
B_CORE = B // N_CORES      # 4096
IBLK = 512                 # rows per block
NSUB = IBLK // 128         # 4 sub-tiles of 128 rows
NBLK = B_CORE // IBLK      # 8
NCHUNK = D // 128          # 24 contraction chunks
NPAIR = NCHUNK // 2        # 12 transpose pairs per block
DDOF = float(D) / float(D - 1)
INV_DM1 = 1.0 / float(D - 1)
SQ_SCALE = float(np.sqrt(1.0 / (D - 1)))

XBUFS = int(os.environ.get("K_XBUFS", "22"))
POST_DEPTH = int(os.environ.get("K_POST_DEPTH", "1"))
DVE_COPIES = int(os.environ.get("K_DVE_COPIES", "9"))   # of 12 per block
DVE_STATS = int(os.environ.get("K_DVE_STATS", "2"))     # of 4 per block
GEMM_LAG = int(os.environ.get("K_GEMM_LAG", "2"))       # pairs of copy lead

LAST_EXEC_NS = None
_CACHE = {}


def _build():
    nc = bacc.Bacc("TRN2", target_bir_lowering=False, debug=False, num_devices=1)

    x_d = nc.dram_tensor("x", [B_CORE, D], F32, kind="ExternalInput").ap()
    w1t_d = nc.dram_tensor("w1t", [128, NCHUNK * M], F16, kind="ExternalInput").ap()
    w2t_d = nc.dram_tensor("w2t", [H, H], F16, kind="ExternalInput").ap()
    w3t_d = nc.dram_tensor("w3t", [H, O], F16, kind="ExternalInput").ap()
    negsm_d = nc.dram_tensor("negsm", [M, M], F16, kind="ExternalInput").ap()
    e32_d = nc.dram_tensor("e32", [M, 1], F16, kind="ExternalInput").ap()
    b1_d = nc.dram_tensor("b1c", [H, 1], F32, kind="ExternalInput").ap()
    b2_d = nc.dram_tensor("b2c", [H, 1], F32, kind="ExternalInput").ap()
    b3_d = nc.dram_tensor("b3c", [O, 1], F32, kind="ExternalInput").ap()
    idh_d = nc.dram_tensor("idh", [128, 128], F16, kind="ExternalInput").ap()
    idf_d = nc.dram_tensor("idf", [128, 128], F32, kind="ExternalInput").ap()
    zf_d = nc.dram_tensor("zrowf", [H, IBLK], F32, kind="ExternalInput").ap()
    y_d = nc.dram_tensor("y", [B_CORE, O], F32, kind="ExternalOutput").ap()

    with tile.TileContext(nc) as tc:
        with tc.tile_pool(name="consts", bufs=1) as consts, \
             tc.tile_pool(name="xpool", bufs=XBUFS) as xpool, \
             tc.tile_pool(name="sqpool", bufs=2) as sqpool, \
             tc.tile_pool(name="xtpool", bufs=4) as xtpool, \
             tc.tile_pool(name="spool", bufs=2) as spool, \
             tc.tile_pool(name="sspool", bufs=12) as sspool, \
             tc.tile_pool(name="opool", bufs=2) as opool, \
             tc.tile_pool(name="pxt", bufs=4, space="PSUM") as pxt_pool, \
             tc.tile_pool(name="py0", bufs=2, space="PSUM") as py0_pool, \
             tc.tile_pool(name="pl", bufs=2, space="PSUM") as pl_pool:

            # ---- constants ----
            w1t_sb = consts.tile([128, NCHUNK, M], F16)
            nc.sync.dma_start(
                out=w1t_sb, in_=w1t_d.rearrange("p (c h) -> p c h", h=M)
            )
            w2t_sb = consts.tile([H, H], F16)
            nc.sync.dma_start(out=w2t_sb, in_=w2t_d)
            w3t_sb = consts.tile([H, O], F16)
            nc.sync.dma_start(out=w3t_sb, in_=w3t_d)
            negsm_sb = consts.tile([M, M], F16)
            nc.sync.dma_start(out=negsm_sb, in_=negsm_d)
            e32_sb = consts.tile([M, 1], F16)
            nc.sync.dma_start(out=e32_sb, in_=e32_d)
            b1_sb = consts.tile([H, 1], F32)
            nc.sync.dma_start(out=b1_sb, in_=b1_d)
            b2_sb = consts.tile([H, 1], F32)
            nc.sync.dma_start(out=b2_sb, in_=b2_d)
            b3_sb = consts.tile([O, 1], F32)
            nc.sync.dma_start(out=b3_sb, in_=b3_d)
            idh_sb = consts.tile([128, 128], F16)
            nc.sync.dma_start(out=idh_sb, in_=idh_d)
            idf_sb = consts.tile([128, 128], F32)
            nc.sync.dma_start(out=idf_sb, in_=idf_d)
            inv32 = consts.tile([H, IBLK], F32)
            nc.sync.dma_start(out=inv32, in_=zf_d)

            def post_head(st):
                """Pre-loop part of the post stage (head-start emission)."""
                _b, py0, ssqs, _r0 = st
                # ssq columns -> one [1, IBLK] psum row  [PE]
                psr = pl_pool.tile([1, IBLK], F32, tag="pl")
                for s in range(NSUB):
                    nc.tensor.matmul(
                        psr[:, s * 128:(s + 1) * 128], ssqs[s], idf_sb,
                        start=True, stop=True,
                    )
                # full psum tile -> sbuf fp16; the one-hot stationaries
                # only read row MP, rows 0..31 are finite y0 values
                mrow33 = spool.tile([M, IBLK], F16, tag="mrow")
                nc.scalar.copy(mrow33, py0)
                return st + (psr, mrow33)

            def post_negs(st):
                _b, py0, _s, _r0, psr, mrow33 = st
                # K=33 one-hot correction: py0 += negsM.T @ mrow33
                nc.tensor.matmul(py0, negsm_sb, mrow33,
                                 start=False, stop=True, skip_group_check=True)
                # mean to partition 0 via one-hot matmul, then m^2 from PSUM
                m0p = pl_pool.tile([1, IBLK], F32, tag="pl")
                nc.tensor.matmul(m0p, e32_sb, mrow33, start=True, stop=True)
                msq = spool.tile([1, IBLK], F32, tag="msq")
                nc.scalar.activation(msq, m0p, AF.Square, scale=1.0)
                # var = ssq/(D-1) - m^2 * D/(D-1);  inv = 1/sqrt(var)
                var_row = spool.tile([1, IBLK], F32, tag="vrow")
                nc.vector.scalar_tensor_tensor(
                    out=var_row, in0=msq, scalar=-DDOF,
                    in1=psr, op0=ALU.mult, op1=ALU.add,
                )
                nc.scalar.activation(inv32[0:1, :], var_row,
                                     AF.Abs_reciprocal_sqrt, scale=1.0)
                inv_b = spool.tile([H, IBLK], F32, tag="invb")
                nc.vector.stream_shuffle(inv_b, inv32, [0] * 32)
                t1 = spool.tile([H, IBLK], F32, tag="t1")
                nc.vector.tensor_mul(t1, py0[0:H, :], inv_b)
                return t1

            def post_l1(st, t1):
                h1 = spool.tile([H, IBLK], F16, tag="h1")
                nc.scalar.activation(h1, t1, AF.Prelu, bias=b1_sb, scale=1.0,
                                     alpha=0.01)
                p2 = pl_pool.tile([H, IBLK], F32, tag="pl")
                nc.tensor.matmul(p2, w2t_sb, h1, start=True, stop=True)
                return p2

            def post_l2(st, p2):
                h2 = spool.tile([H, IBLK], F16, tag="h2")
                nc.scalar.activation(h2, p2, AF.Prelu, bias=b2_sb, scale=1.0,
                                     alpha=0.01)
                p3 = pl_pool.tile([O, IBLK], F32, tag="pl")
                nc.tensor.matmul(p3, w3t_sb, h2, start=True, stop=True)
                return p3

            def post_l3(st, p3):
                y3 = spool.tile([O, IBLK], F32, tag="y3")
                nc.scalar.activation(y3, p3, AF.Prelu, bias=b3_sb, scale=1.0,
                                     alpha=0.01)
                return y3

            def post_out(st, y3):
                _b, _py0, _s, r0 = st[:4]
                pout = pl_pool.tile([128, NSUB, O], F32, tag="pl")
                for s in range(NSUB):
                    nc.tensor.transpose(
                        pout[:, s, :],
                        y3[:, s * 128:(s + 1) * 128],
                        idf_sb[0:O, 0:O],
                    )
                out_sb = opool.tile([128, NSUB, O], F32, tag="out")
                nc.vector.tensor_copy(out_sb, pout)
                nc.sync.dma_start(
                    out=y_d[r0:r0 + IBLK, :].rearrange("(s p) c -> p s c", p=128),
                    in_=out_sb,
                )

            from collections import deque
            posts = deque()
            for b in range(NBLK):
                r0 = b * IBLK
                # ---- load x block (fp32 -> fp16 cast in DMA) ----
                xs = []
                for s in range(NSUB):
                    xt = xpool.tile([128, D], F16, tag="xnat")
                    nc.gpsimd.dma_start(
                        out=xt, in_=x_d[r0 + s * 128:r0 + (s + 1) * 128, :]
                    )
                    xs.append(xt)

                post = posts.popleft() if len(posts) >= POST_DEPTH else None
                t1 = p2 = p3 = y3 = None

                # ---- per-block state ----
                ssqs = [None] * NSUB
                n_dve_stat = 0
                n_act_stat = 0

                def emit_stat(s, on_dve):
                    xsq = sqpool.tile([128, D], F16, tag="xsq")
                    ssq = sspool.tile([128, 1], F32, tag="ssq")
                    if on_dve:
                        nc.vector.scalar_tensor_tensor(
                            out=xsq, in0=xs[s], scalar=INV_DM1, in1=xs[s],
                            op0=ALU.mult, op1=ALU.mult, accum_out=ssq,
                        )
                    else:
                        nc.scalar.activation(
                            xsq, xs[s], AF.Square, scale=SQ_SCALE,
                            accum_out=ssq,
                        )
                    ssqs[s] = ssq

                # ---- transpose x + stream against the w1t stationary ----
                py0 = py0_pool.tile([M, IBLK], F32)
                prevs = []
                for c2 in range(NPAIR):
                    pxt = pxt_pool.tile([128, 2 * IBLK], F16)
                    for q in range(2):
                        c = 2 * c2 + q
                        for s in range(NSUB):
                            nc.tensor.transpose(
                                pxt[:, q * IBLK + s * 128:q * IBLK + (s + 1) * 128],
                                xs[s][:, c * 128:(c + 1) * 128],
                                idh_sb,
                            )
                    xts = xtpool.tile([128, 2 * IBLK], F16, tag="xt")
                    if c2 < DVE_COPIES:
                        nc.vector.tensor_copy(xts, pxt)
                    else:
                        nc.scalar.copy(xts, pxt)
                    prevs.append((c2, xts))
                    if len(prevs) > GEMM_LAG:
                        pc2, pxts = prevs.pop(0)
                        for q in range(2):
                            c = 2 * pc2 + q
                            nc.tensor.matmul(
                                py0, w1t_sb[:, c, :],
                                pxts[:, q * IBLK:(q + 1) * IBLK],
                                start=(c == 0), stop=False,
                            )

                    # interleaved stats (keep early DVE copies early)
                    if c2 in (2, 4) and n_dve_stat < DVE_STATS:
                        emit_stat(n_dve_stat + n_act_stat, True)
                        n_dve_stat += 1
                    if c2 in (6, 8) and n_act_stat < NSUB - DVE_STATS:
                        emit_stat(n_dve_stat + n_act_stat, False)
                        n_act_stat += 1

                    # interleaved post stage for the previous block
                    if post is not None:
                        if c2 == 1:
                            t1 = post_negs(post)
                        elif c2 == 3:
                            p2 = post_l1(post, t1)
                        elif c2 == 5:
                            p3 = post_l2(post, p2)
                        elif c2 == 7:
                            y3 = post_l3(post, p3)
                        elif c2 == 9:
                            post_out(post, y3)

                while n_dve_stat < DVE_STATS:
                    emit_stat(n_dve_stat + n_act_stat, True)
                    n_dve_stat += 1
                while n_act_stat < NSUB - DVE_STATS:
                    emit_stat(n_dve_stat + n_act_stat, False)
                    n_act_stat += 1

                for pc2, pxts in prevs:
                    for q in range(2):
                        c = 2 * pc2 + q
                        nc.tensor.matmul(
                            py0, w1t_sb[:, c, :],
                            pxts[:, q * IBLK:(q + 1) * IBLK],
                            start=(c == 0), stop=(c == NCHUNK - 1),
                        )

                posts.append(post_head((b, py0, ssqs, r0)))

            # drain the remaining post stages
            while posts:
                post = posts.popleft()
                t1 = post_negs(post)
                p2 = post_l1(post, t1)
                p3 = post_l2(post, p2)
                y3 = post_l3(post, p3)
                post_out(post, y3)

    nc.compile()
    return nc


def _prep_inputs(x, w1, b1, w2, b2, w3, b3):
    x = np.ascontiguousarray(np.asarray(x, dtype=np.float32))
    w1 = np.asarray(w1, dtype=np.float32)
    w2 = np.asarray(w2, dtype=np.float32)
    w3 = np.asarray(w3, dtype=np.float32)
    b1 = np.asarray(b1, dtype=np.float32)
    b2 = np.asarray(b2, dtype=np.float32)
    b3 = np.asarray(b3, dtype=np.float32)

    # augmented stationary: cols 0..31 = w1^T, col 32 = 1/D (mean)
    w1a = np.zeros((D, M), dtype=np.float32)
    w1a[:, 0:H] = w1.T
    w1a[:, MP] = 1.0 / D
    negsm = np.zeros((M, M), dtype=np.float32)
    negsm[MP, 0:H] = -w1.astype(np.float64).sum(axis=1)
    e32 = np.zeros((M, 1), dtype=np.float32)
    e32[MP, 0] = 1.0

    common = {
        # [128, 24*33]: partition p holds w1a[c*128+p, :] for each chunk c
        "w1t": np.ascontiguousarray(
            w1a.reshape(NCHUNK, 128, M).transpose(1, 0, 2).reshape(128, NCHUNK * M)
        ).astype(np.float16),
        "w2t": np.ascontiguousarray(w2.T).astype(np.float16),
        "w3t": np.ascontiguousarray(w3.T).astype(np.float16),
        "negsm": np.ascontiguousarray(negsm).astype(np.float16),
        "e32": np.ascontiguousarray(e32).astype(np.float16),
        "b1c": np.ascontiguousarray(b1[:, None]),
        "b2c": np.ascontiguousarray(b2[:, None]),
        "b3c": np.ascontiguousarray(b3[:, None]),
        "idh": np.eye(128, dtype=np.float16),
        "idf": np.eye(128, dtype=np.float32),
        "zrowf": np.zeros((H, IBLK), dtype=np.float32),
    }
    in_maps = []
    for c in range(N_CORES):
        m = dict(common)
        m["x"] = x[c * B_CORE:(c + 1) * B_CORE]
        in_maps.append(m)
    return in_maps


def kernel(x, w1, b1, w2, b2, w3, b3):
    global LAST_EXEC_NS
    if "nc" not in _CACHE:
        _CACHE["nc"] = _build()
    nc = _CACHE["nc"]
    in_maps = _prep_inputs(x, w1, b1, w2, b2, w3, b3)
    trace = bool(int(os.environ.get("KERNEL_PROFILE", "0")))
    res = run_bass_kernel_spmd(nc, in_maps, core_ids=list(range(N_CORES)),
                               trace=trace)
    LAST_EXEC_NS = res.exec_time_ns
    out = np.concatenate([r["y"] for r in res.results], axis=0)
    return out.astype(np.float32)


# revision 25
# speedup vs baseline: 1.0815x; 1.0815x over previous
"""Trainium2 Bass kernel for nn_NeuralNet_19250043421419.

Row-normalize x (mean/std over D=3072, ddof=1) then a 3-layer MLP
(3072->32->32->10) with LeakyReLU(0.01) after every layer.

Strategy: pure data parallel over 8 NeuronCores (batch 32768 -> 4096/core).
Per core, per 512-row block:
  - DMA x in natural layout, casting fp32->fp16 in the SWDGE DMA.  GpSimd
    does nothing else, so DMA issue is never gated by compute.
  - Per-row sum(x^2)/(D-1) split between DVE (scalar_tensor_tensor with
    accum) and ACT (Square activation with accum), tunable.
  - PE transposes x into [d, i] tiles with the dedicated transpose datapath
    (fp16 PSUM output: halves both PSUM banks and copy cost); the
    PSUM->SBUF copies are split between DVE (fp16 2x rate) and ACT.
  - PE streams the transposed tiles against [w1^T | 1/D] (M=33),
    accumulating y0_raw in PSUM rows 0..31 and the row-mean in row 32.
  - Normalization folds in afterwards: (x-m)/s @ w1^T =
    (y0_raw - m * rowsum(w1)) / s.  The mean row moves to partition 0 via
    a small SBUF->SBUF HWDGE DMA, the K=1 mean-correction matmul
    accumulates onto the closed PSUM group, and the 1/s scaling is a DVE
    multiply against a stream_shuffle-broadcast row, with
    var = ssq/(D-1) - m^2 * D/(D-1) (Bessel).
  - The post stage for block b-1 is interleaved into block b's emission so
    every engine sees its dependencies ready on arrival (PE post matmuls
    are spread between transpose pairs).
  - PE transposes the [10, 512] result back to natural [512, 10] and DMAs out.
"""
import os
import sys

for _p in ("/opt/trn_rl_repo", "/root/.axon_site/_ro/trn_rl_repo"):
    if os.path.isdir(_p) and _p not in sys.path:
        sys.path.append(_p)

import numpy as np

import concourse.bass as bass
import concourse.bacc as bacc
import concourse.tile as tile
from concourse import mybir
from concourse.bass_utils import run_bass_kernel_spmd

F32 = mybir.dt.float32
F16 = mybir.dt.float16
AF = mybir.ActivationFunctionType
ALU = mybir.AluOpType

N_CORES = 8
B = 32768
D = 3072
H = 32
O = 10
M = 33                     # GEMM stationary cols: [w1^T | 1/D]
MP = 32                    # partition of the mean row
B_CORE = B // N_CORES      # 4096
IBLK = 512                 # rows per block
NSUB = IBLK // 128         # 4 sub-tiles of 128 rows
NBLK = B_CORE // IBLK      # 8
NCHUNK = D // 128          # 24 contraction chunks
NPAIR = NCHUNK // 2        # 12 transpose pairs per block
DDOF = float(D) / float(D - 1)
INV_DM1 = 1.0 / float(D - 1)
SQ_SCALE = float(np.sqrt(1.0 / (D - 1)))

XBUFS = int(os.environ.get("K_XBUFS", "22"))
POST_DEPTH = int(os.environ.get("K_POST_DEPTH", "1"))
DVE_COPIES = int(os.environ.get("K_DVE_COPIES", "9"))   # of 12 per block
DVE_STATS = int(os.environ.get("K_DVE_STATS", "2"))     # of 4 per block
GEMM_LAG = int(os.environ.get("K_GEMM_LAG", "1"))       # pairs of copy lead

LAST_EXEC_NS = None
_CACHE = {}


def _build():
    nc = bacc.Bacc("TRN2", target_bir_lowering=False, debug=False, num_devices=1)

    x_d = nc.dram_tensor("x", [B_CORE, D], F32, kind="ExternalInput").ap()
    w1t_d = nc.dram_tensor("w1t", [128, NCHUNK * M], F16, kind="ExternalInput").ap()
    w2t_d = nc.dram_tensor("w2t", [H, H], F16, kind="ExternalInput").ap()
    w3t_d = nc.dram_tensor("w3t", [H, O], F16, kind="ExternalInput").ap()
    negsm_d = nc.dram_tensor("negsm", [M, M], F16, kind="ExternalInput").ap()
    e32_d = nc.dram_tensor("e32", [M, 1], F16, kind="ExternalInput").ap()
    b1_d = nc.dram_tensor("b1c", [H, 1], F32, kind="ExternalInput").ap()
    b2_d = nc.dram_tensor("b2c", [H, 1], F32, kind="ExternalInput").ap()
    b3_d = nc.dram_tensor("b3c", [O, 1], F32, kind="ExternalInput").ap()
    idh_d = nc.dram_tensor("idh", [128, 128], F16, kind="ExternalInput").ap()
    idf_d = nc.dram_tensor("idf", [128, 128], F32, kind="ExternalInput").ap()
    zf_d = nc.dram_tensor("zrowf", [H, IBLK], F32, kind="ExternalInput").ap()
    y_d = nc.dram_tensor("y", [B_CORE, O], F32, kind="ExternalOutput").ap()

    with tile.TileContext(nc) as tc:
        with tc.tile_pool(name="consts", bufs=1) as consts, \
             tc.tile_pool(name="xpool", bufs=XBUFS) as xpool, \
             tc.tile_pool(name="sqpool", bufs=2) as sqpool, \
             tc.tile_pool(name="xtpool", bufs=4) as xtpool, \
             tc.tile_pool(name="spool", bufs=2) as spool, \
             tc.tile_pool(name="sspool", bufs=12) as sspool, \
             tc.tile_pool(name="opool", bufs=2) as opool, \
             tc.tile_pool(name="pxt", bufs=4, space="PSUM") as pxt_pool, \
             tc.tile_pool(name="py0", bufs=2, space="PSUM") as py0_pool, \
             tc.tile_pool(name="pl", bufs=2, space="PSUM") as pl_pool:

            # ---- constants ----
            w1t_sb = consts.tile([128, NCHUNK, M], F16)
            nc.sync.dma_start(
                out=w1t_sb, in_=w1t_d.rearrange("p (c h) -> p c h", h=M)
            )
            w2t_sb = consts.tile([H, H], F16)
            nc.sync.dma_start(out=w2t_sb, in_=w2t_d)
            w3t_sb = consts.tile([H, O], F16)
            nc.sync.dma_start(out=w3t_sb, in_=w3t_d)
            negsm_sb = consts.tile([M, M], F16)
            nc.sync.dma_start(out=negsm_sb, in_=negsm_d)
            e32_sb = consts.tile([M, 1], F16)
            nc.sync.dma_start(out=e32_sb, in_=e32_d)
            b1_sb = consts.tile([H, 1], F32)
            nc.sync.dma_start(out=b1_sb, in_=b1_d)
            b2_sb = consts.tile([H, 1], F32)
            nc.sync.dma_start(out=b2_sb, in_=b2_d)
            b3_sb = consts.tile([O, 1], F32)
            nc.sync.dma_start(out=b3_sb, in_=b3_d)
            idh_sb = consts.tile([128, 128], F16)
            nc.sync.dma_start(out=idh_sb, in_=idh_d)
            idf_sb = consts.tile([128, 128], F32)
            nc.sync.dma_start(out=idf_sb, in_=idf_d)
            inv32 = consts.tile([H, IBLK], F32)
            nc.sync.dma_start(out=inv32, in_=zf_d)

            def post_head(st):
                """Pre-loop part of the post stage (head-start emission)."""
                _b, py0, ssqs, _r0 = st
                # ssq columns -> one [1, IBLK] psum row  [PE]
                psr = pl_pool.tile([1, IBLK], F32, tag="pl")
                for s in range(NSUB):
                    nc.tensor.matmul(
                        psr[:, s * 128:(s + 1) * 128], ssqs[s], idf_sb,
                        start=True, stop=True,
                    )
                # full psum tile -> sbuf fp16; the one-hot stationaries
                # only read row MP, rows 0..31 are finite y0 values
                mrow33 = spool.tile([M, IBLK], F16, tag="mrow")
                nc.scalar.copy(mrow33, py0)
                return st + (psr, mrow33)

            def post_negs(st):
                _b, py0, _s, _r0, psr, mrow33 = st
                # K=33 one-hot correction: py0 += negsM.T @ mrow33
                nc.tensor.matmul(py0, negsm_sb, mrow33,
                                 start=False, stop=True, skip_group_check=True)
                # mean to partition 0 via one-hot matmul, then m^2 from PSUM
                m0p = pl_pool.tile([1, IBLK], F32, tag="pl")
                nc.tensor.matmul(m0p, e32_sb, mrow33, start=True, stop=True)
                msq = spool.tile([1, IBLK], F32, tag="msq")
                nc.scalar.activation(msq, m0p, AF.Square, scale=1.0)
                # var = ssq/(D-1) - m^2 * D/(D-1);  inv = 1/sqrt(var)
                var_row = spool.tile([1, IBLK], F32, tag="vrow")
                nc.vector.scalar_tensor_tensor(
                    out=var_row, in0=msq, scalar=-DDOF,
                    in1=psr, op0=ALU.mult, op1=ALU.add,
                )
                nc.scalar.activation(inv32[0:1, :], var_row,
                                     AF.Abs_reciprocal_sqrt, scale=1.0)
                inv_b = spool.tile([H, IBLK], F32, tag="invb")
                nc.vector.stream_shuffle(inv_b, inv32, [0] * 32)
                t1 = spool.tile([H, IBLK], F32, tag="t1")
                nc.vector.tensor_mul(t1, py0[0:H, :], inv_b)
                return t1

            def post_l1(st, t1):
                h1 = spool.tile([H, IBLK], F16, tag="h1")
                nc.scalar.activation(h1, t1, AF.Prelu, bias=b1_sb, scale=1.0,
                                     alpha=0.01)
                p2 = pl_pool.tile([H, IBLK], F32, tag="pl")
                nc.tensor.matmul(p2, w2t_sb, h1, start=True, stop=True)
                return p2

            def post_l2(st, p2):
                h2 = spool.tile([H, IBLK], F16, tag="h2")
                nc.scalar.activation(h2, p2, AF.Prelu, bias=b2_sb, scale=1.0,
                                     alpha=0.01)
                p3 = pl_pool.tile([O, IBLK], F32, tag="pl")
                nc.tensor.matmul(p3, w3t_sb, h2, start=True, stop=True)
                return p3

            def post_l3(st, p3):
                y3 = spool.tile([O, IBLK], F32, tag="y3")
                nc.scalar.activation(y3, p3, AF.Prelu, bias=b3_sb, scale=1.0,
                                     alpha=0.01)
                return y3

            def post_out(st, y3):
                _b, _py0, _s, r0 = st[:4]
                pout = pl_pool.tile([128, NSUB, O], F32, tag="pl")
                for s in range(NSUB):
                    nc.tensor.transpose(
                        pout[:, s, :],
                        y3[:, s * 128:(s + 1) * 128],
                        idf_sb[0:O, 0:O],
                    )
                out_sb = opool.tile([128, NSUB, O], F32, tag="out")
                nc.vector.tensor_copy(out_sb, pout)
                nc.sync.dma_start(
                    out=y_d[r0:r0 + IBLK, :].rearrange("(s p) c -> p s c", p=128),
                    in_=out_sb,
                )

            from collections import deque
            posts = deque()
            for b in range(NBLK):
                r0 = b * IBLK
                # ---- load x block (fp32 -> fp16 cast in DMA) ----
                xs = []
                for s in range(NSUB):
                    xt = xpool.tile([128, D], F16, tag="xnat")
                    nc.gpsimd.dma_start(
                        out=xt, in_=x_d[r0 + s * 128:r0 + (s + 1) * 128, :]
                    )
                    xs.append(xt)

                post = posts.popleft() if len(posts) >= POST_DEPTH else None
                t1 = p2 = p3 = y3 = None

                # ---- per-block state ----
                ssqs = [None] * NSUB
                n_dve_stat = 0
                n_act_stat = 0

                def emit_stat(s, on_dve):
                    xsq = sqpool.tile([128, D], F16, tag="xsq")
                    ssq = sspool.tile([128, 1], F32, tag="ssq")
                    if on_dve:
                        nc.vector.scalar_tensor_tensor(
                            out=xsq, in0=xs[s], scalar=INV_DM1, in1=xs[s],
                            op0=ALU.mult, op1=ALU.mult, accum_out=ssq,
                        )
                    else:
                        nc.scalar.activation(
                            xsq, xs[s], AF.Square, scale=SQ_SCALE,
                            accum_out=ssq,
                        )
                    ssqs[s] = ssq

                # ---- transpose x + stream against the w1t stationary ----
                py0 = py0_pool.tile([M, IBLK], F32)
                prevs = []
                for c2 in range(NPAIR):
                    pxt = pxt_pool.tile([128, 2 * IBLK], F16)
                    for q in range(2):
                        c = 2 * c2 + q
                        for s in range(NSUB):
                            nc.tensor.transpose(
                                pxt[:, q * IBLK + s * 128:q * IBLK + (s + 1) * 128],
                                xs[s][:, c * 128:(c + 1) * 128],
                                idh_sb,
                            )
                    xts = xtpool.tile([128, 2 * IBLK], F16, tag="xt")
                    if c2 < DVE_COPIES:
                        nc.vector.tensor_copy(xts, pxt)
                    else:
                        nc.scalar.copy(xts, pxt)
                    prevs.append((c2, xts))
                    if len(prevs) > GEMM_LAG:
                        pc2, pxts = prevs.pop(0)
                        for q in range(2):
                            c = 2 * pc2 + q
                            nc.tensor.matmul(
                                py0, w1t_sb[:, c, :],
                                pxts[:, q * IBLK:(q + 1) * IBLK],
                                start=(c == 0), stop=False,
                            )

                    # interleaved stats (keep early DVE copies early)
                    if c2 in (2, 4) and n_dve_stat < DVE_STATS:
                        emit_stat(n_dve_stat + n_act_stat, True)
                        n_dve_stat += 1
                    if c2 in (6, 8) and n_act_stat < NSUB - DVE_STATS:
                        emit_stat(n_dve_stat + n_act_stat, False)
                        n_act_stat += 1

                    # interleaved post stage for the previous block
                    if post is not None:
                        if c2 == 1:
                            t1 = post_negs(post)
                        elif c2 == 3:
                            p2 = post_l1(post, t1)
                        elif c2 == 5:
                            p3 = post_l2(post, p2)
                        elif c2 == 7:
                            y3 = post_l3(post, p3)
                        elif c2 == 9:
                            post_out(post, y3)

                while n_dve_stat < DVE_STATS:
                    emit_stat(n_dve_stat + n_act_stat, True)
                    n_dve_stat += 1
                while n_act_stat < NSUB - DVE_STATS:
                    emit_stat(n_dve_stat + n_act_stat, False)
                    n_act_stat += 1

                for pc2, pxts in prevs:
                    for q in range(2):
                        c = 2 * pc2 + q
                        nc.tensor.matmul(
                            py0, w1t_sb[:, c, :],
                            pxts[:, q * IBLK:(q + 1) * IBLK],
                            start=(c == 0), stop=(c == NCHUNK - 1),
                        )

                posts.append(post_head((b, py0, ssqs, r0)))

            # drain the remaining post stages
            while posts:
                post = posts.popleft()
                t1 = post_negs(post)
                p2 = post_l1(post, t1)
                p3 = post_l2(post, p2)
                y3 = post_l3(post, p3)
                post_out(post, y3)

    nc.compile()
    return nc


def _prep_inputs(x, w1, b1, w2, b2, w3, b3):
    x = np.ascontiguousarray(np.asarray(x, dtype=np.float32))
    w1 = np.asarray(w1, dtype=np.float32)
    w2 = np.asarray(w2, dtype=np.float32)
    w3 = np.asarray(w3, dtype=np.float32)
    b1 = np.asarray(b1, dtype=np.float32)
    b2 = np.asarray(b2, dtype=np.float32)
    b3 = np.asarray(b3, dtype=np.float32)

    # augmented stationary: cols 0..31 = w1^T, col 32 = 1/D (mean)
    w1a = np.zeros((D, M), dtype=np.float32)
    w1a[:, 0:H] = w1.T
    w1a[:, MP] = 1.0 / D
    negsm = np.zeros((M, M), dtype=np.float32)
    negsm[MP, 0:H] = -w1.astype(np.float64).sum(axis=1)
    e32 = np.zeros((M, 1), dtype=np.float32)
    e32[MP, 0] = 1.0

    common = {
        # [128, 24*33]: partition p holds w1a[c*128+p, :] for each chunk c
        "w1t": np.ascontiguousarray(
            w1a.reshape(NCHUNK, 128, M).transpose(1, 0, 2).reshape(128, NCHUNK * M)
        ).astype(np.float16),
        "w2t": np.ascontiguousarray(w2.T).astype(np.float16),
        "w3t": np.ascontiguousarray(w3.T).astype(np.float16),
        "negsm": np.ascontiguousarray(negsm).astype(np.float16),
        "e32": np.ascontiguousarray(e32).astype(np.float16),
        "b1c": np.ascontiguousarray(b1[:, None]),
        "b2c": np.ascontiguousarray(b2[:, None]),
        "b3c": np.ascontiguousarray(b3[:, None]),
        "idh": np.eye(128, dtype=np.float16),
        "idf": np.eye(128, dtype=np.float32),
        "zrowf": np.zeros((H, IBLK), dtype=np.float32),
    }
    in_maps = []
    for c in range(N_CORES):
        m = dict(common)
        m["x"] = x[c * B_CORE:(c + 1) * B_CORE]
        in_maps.append(m)
    return in_maps


def kernel(x, w1, b1, w2, b2, w3, b3):
    global LAST_EXEC_NS
    if "nc" not in _CACHE:
        _CACHE["nc"] = _build()
    nc = _CACHE["nc"]
    in_maps = _prep_inputs(x, w1, b1, w2, b2, w3, b3)
    trace = bool(int(os.environ.get("KERNEL_PROFILE", "0")))
    res = run_bass_kernel_spmd(nc, in_maps, core_ids=list(range(N_CORES)),
                               trace=trace)
    LAST_EXEC_NS = res.exec_time_ns
    out = np.concatenate([r["y"] for r in res.results], axis=0)
    return out.astype(np.float32)


# revision 26
# speedup vs baseline: 1.0903x; 1.0081x over previous
"""Trainium2 Bass kernel for nn_NeuralNet_19250043421419.

Row-normalize x (mean/std over D=3072, ddof=1) then a 3-layer MLP
(3072->32->32->10) with LeakyReLU(0.01) after every layer.

Strategy: pure data parallel over 8 NeuronCores (batch 32768 -> 4096/core).
Per core, per 512-row block:
  - DMA x in natural layout, casting fp32->fp16 in the SWDGE DMA.  GpSimd
    does nothing else, so DMA issue is never gated by compute.
  - Per-row sum(x^2)/(D-1) split between DVE (scalar_tensor_tensor with
    accum) and ACT (Square activation with accum), tunable.
  - PE transposes x into [d, i] tiles with the dedicated transpose datapath
    (fp16 PSUM output: halves both PSUM banks and copy cost); the
    PSUM->SBUF copies are split between DVE (fp16 2x rate) and ACT.
  - PE streams the transposed tiles against [w1^T | 1/D] (M=33),
    accumulating y0_raw in PSUM rows 0..31 and the row-mean in row 32.
  - Normalization folds in afterwards: (x-m)/s @ w1^T =
    (y0_raw - m * rowsum(w1)) / s.  The mean correction is a one-hot K=33
    matmul (stationary rows 0..31 zero) accumulated onto the closed PSUM
    group; a second one-hot matmul lands the mean on partition 0 where ACT
    squares it straight out of PSUM, and the 1/s scaling is a DVE multiply
    against a stream_shuffle-broadcast row, with
    var = ssq/(D-1) - m^2 * D/(D-1) (Bessel).
  - The post stage for block b-1 is interleaved into block b's emission so
    every engine sees its dependencies ready on arrival (PE post matmuls
    are spread between transpose pairs).
  - PE transposes the [10, 512] result back to natural [512, 10] and DMAs out.
"""
import os
import sys

for _p in ("/opt/trn_rl_repo", "/root/.axon_site/_ro/trn_rl_repo"):
    if os.path.isdir(_p) and _p not in sys.path:
        sys.path.append(_p)

import numpy as np

import concourse.bass as bass
import concourse.bacc as bacc
import concourse.tile as tile
from concourse import mybir
from concourse.bass_utils import run_bass_kernel_spmd

F32 = mybir.dt.float32
F16 = mybir.dt.float16
AF = mybir.ActivationFunctionType
ALU = mybir.AluOpType

N_CORES = 8
B = 32768
D = 3072
H = 32
O = 10
M = 33                     # GEMM stationary cols: [w1^T | 1/D]
MP = 32                    # partition of the mean row
B_CORE = B // N_CORES      # 4096
IBLK = 512                 # rows per block
NSUB = IBLK // 128         # 4 sub-tiles of 128 rows
NBLK = B_CORE // IBLK      # 8
NCHUNK = D // 128          # 24 contraction chunks
NPAIR = NCHUNK // 2        # 12 transpose pairs per block
DDOF = float(D) / float(D - 1)
INV_DM1 = 1.0 / float(D - 1)
SQ_SCALE = float(np.sqrt(1.0 / (D - 1)))

XBUFS = int(os.environ.get("K_XBUFS", "22"))
POST_DEPTH = int(os.environ.get("K_POST_DEPTH", "1"))
DVE_COPIES = int(os.environ.get("K_DVE_COPIES", "9"))   # of 12 per block
DVE_STATS = int(os.environ.get("K_DVE_STATS", "2"))     # of 4 per block
GEMM_LAG = int(os.environ.get("K_GEMM_LAG", "1"))       # pairs of copy lead

LAST_EXEC_NS = None
_CACHE = {}


def _build():
    nc = bacc.Bacc("TRN2", target_bir_lowering=False, debug=False, num_devices=1)

    x_d = nc.dram_tensor("x", [B_CORE, D], F32, kind="ExternalInput").ap()
    w1t_d = nc.dram_tensor("w1t", [128, NCHUNK * M], F16, kind="ExternalInput").ap()
    w2t_d = nc.dram_tensor("w2t", [H, H], F16, kind="ExternalInput").ap()
    w3t_d = nc.dram_tensor("w3t", [H, O], F16, kind="ExternalInput").ap()
    negsm_d = nc.dram_tensor("negsm", [M, M], F16, kind="ExternalInput").ap()
    e32_d = nc.dram_tensor("e32", [M, 1], F16, kind="ExternalInput").ap()
    b1_d = nc.dram_tensor("b1c", [H, 1], F32, kind="ExternalInput").ap()
    b2_d = nc.dram_tensor("b2c", [H, 1], F32, kind="ExternalInput").ap()
    b3_d = nc.dram_tensor("b3c", [O, 1], F32, kind="ExternalInput").ap()
    idh_d = nc.dram_tensor("idh", [128, 128], F16, kind="ExternalInput").ap()
    idf_d = nc.dram_tensor("idf", [128, 128], F32, kind="ExternalInput").ap()
    zf_d = nc.dram_tensor("zrowf", [H, IBLK], F32, kind="ExternalInput").ap()
    y_d = nc.dram_tensor("y", [B_CORE, O], F32, kind="ExternalOutput").ap()

    with tile.TileContext(nc) as tc:
        with tc.tile_pool(name="consts", bufs=1) as consts, \
             tc.tile_pool(name="xpool", bufs=XBUFS) as xpool, \
             tc.tile_pool(name="sqpool", bufs=2) as sqpool, \
             tc.tile_pool(name="xtpool", bufs=4) as xtpool, \
             tc.tile_pool(name="spool", bufs=2) as spool, \
             tc.tile_pool(name="sspool", bufs=12) as sspool, \
             tc.tile_pool(name="opool", bufs=2) as opool, \
             tc.tile_pool(name="pxt", bufs=4, space="PSUM") as pxt_pool, \
             tc.tile_pool(name="py0", bufs=2, space="PSUM") as py0_pool, \
             tc.tile_pool(name="pl", bufs=2, space="PSUM") as pl_pool:

            # ---- constants ----
            w1t_sb = consts.tile([128, NCHUNK, M], F16)
            nc.sync.dma_start(
                out=w1t_sb, in_=w1t_d.rearrange("p (c h) -> p c h", h=M)
            )
            w2t_sb = consts.tile([H, H], F16)
            nc.sync.dma_start(out=w2t_sb, in_=w2t_d)
            w3t_sb = consts.tile([H, O], F16)
            nc.sync.dma_start(out=w3t_sb, in_=w3t_d)
            negsm_sb = consts.tile([M, M], F16)
            nc.sync.dma_start(out=negsm_sb, in_=negsm_d)
            e32_sb = consts.tile([M, 1], F16)
            nc.sync.dma_start(out=e32_sb, in_=e32_d)
            b1_sb = consts.tile([H, 1], F32)
            nc.sync.dma_start(out=b1_sb, in_=b1_d)
            b2_sb = consts.tile([H, 1], F32)
            nc.sync.dma_start(out=b2_sb, in_=b2_d)
            b3_sb = consts.tile([O, 1], F32)
            nc.sync.dma_start(out=b3_sb, in_=b3_d)
            idh_sb = consts.tile([128, 128], F16)
            nc.sync.dma_start(out=idh_sb, in_=idh_d)
            idf_sb = consts.tile([128, 128], F32)
            nc.sync.dma_start(out=idf_sb, in_=idf_d)
            inv32 = consts.tile([H, IBLK], F32)
            nc.sync.dma_start(out=inv32, in_=zf_d)

            def post_head(st):
                """Pre-loop part of the post stage (head-start emission)."""
                _b, py0, ssqs, _r0 = st
                # ssq columns -> one [1, IBLK] psum row  [PE]
                psr = pl_pool.tile([1, IBLK], F32, tag="pl")
                for s in range(NSUB):
                    nc.tensor.matmul(
                        psr[:, s * 128:(s + 1) * 128], ssqs[s], idf_sb,
                        start=True, stop=True,
                    )
                # full psum tile -> sbuf fp16; the one-hot stationaries
                # only read row MP, rows 0..31 are finite y0 values
                mrow33 = spool.tile([M, IBLK], F16, tag="mrow")
                nc.scalar.copy(mrow33, py0)
                return st + (psr, mrow33)

            def post_negs(st):
                _b, py0, _s, _r0, psr, mrow33 = st
                # K=33 one-hot correction: py0 += negsM.T @ mrow33
                nc.tensor.matmul(py0, negsm_sb, mrow33,
                                 start=False, stop=True, skip_group_check=True)
                # mean to partition 0 via one-hot matmul, then m^2 from PSUM
                m0p = pl_pool.tile([1, IBLK], F32, tag="pl")
                nc.tensor.matmul(m0p, e32_sb, mrow33, start=True, stop=True)
                msq = spool.tile([1, IBLK], F32, tag="msq")
                nc.scalar.activation(msq, m0p, AF.Square, scale=1.0)
                # var = ssq/(D-1) - m^2 * D/(D-1);  inv = 1/sqrt(var)
                var_row = spool.tile([1, IBLK], F32, tag="vrow")
                nc.vector.scalar_tensor_tensor(
                    out=var_row, in0=msq, scalar=-DDOF,
                    in1=psr, op0=ALU.mult, op1=ALU.add,
                )
                nc.scalar.activation(inv32[0:1, :], var_row,
                                     AF.Abs_reciprocal_sqrt, scale=1.0)
                inv_b = spool.tile([H, IBLK], F32, tag="invb")
                nc.vector.stream_shuffle(inv_b, inv32, [0] * 32)
                t1 = spool.tile([H, IBLK], F32, tag="t1")
                nc.vector.tensor_mul(t1, py0[0:H, :], inv_b)
                return t1

            def post_l1(st, t1):
                h1 = spool.tile([H, IBLK], F16, tag="h1")
                nc.scalar.activation(h1, t1, AF.Prelu, bias=b1_sb, scale=1.0,
                                     alpha=0.01)
                p2 = pl_pool.tile([H, IBLK], F32, tag="pl")
                nc.tensor.matmul(p2, w2t_sb, h1, start=True, stop=True)
                return p2

            def post_l2(st, p2):
                h2 = spool.tile([H, IBLK], F16, tag="h2")
                nc.scalar.activation(h2, p2, AF.Prelu, bias=b2_sb, scale=1.0,
                                     alpha=0.01)
                p3 = pl_pool.tile([O, IBLK], F32, tag="pl")
                nc.tensor.matmul(p3, w3t_sb, h2, start=True, stop=True)
                return p3

            def post_l3(st, p3):
                y3 = spool.tile([O, IBLK], F32, tag="y3")
                nc.scalar.activation(y3, p3, AF.Prelu, bias=b3_sb, scale=1.0,
                                     alpha=0.01)
                return y3

            def post_out(st, y3):
                _b, _py0, _s, r0 = st[:4]
                pout = pl_pool.tile([128, NSUB, O], F32, tag="pl")
                for s in range(NSUB):
                    nc.tensor.transpose(
                        pout[:, s, :],
                        y3[:, s * 128:(s + 1) * 128],
                        idf_sb[0:O, 0:O],
                    )
                out_sb = opool.tile([128, NSUB, O], F32, tag="out")
                nc.vector.tensor_copy(out_sb, pout)
                nc.sync.dma_start(
                    out=y_d[r0:r0 + IBLK, :].rearrange("(s p) c -> p s c", p=128),
                    in_=out_sb,
                )

            from collections import deque
            posts = deque()
            for b in range(NBLK):
                r0 = b * IBLK
                # ---- load x block (fp32 -> fp16 cast in DMA) ----
                xs = []
                for s in range(NSUB):
                    xt = xpool.tile([128, D], F16, tag="xnat")
                    nc.gpsimd.dma_start(
                        out=xt, in_=x_d[r0 + s * 128:r0 + (s + 1) * 128, :]
                    )
                    xs.append(xt)

                post = posts.popleft() if len(posts) >= POST_DEPTH else None
                t1 = p2 = p3 = y3 = None

                # ---- per-block state ----
                ssqs = [None] * NSUB
                n_dve_stat = 0
                n_act_stat = 0

                def emit_stat(s, on_dve):
                    xsq = sqpool.tile([128, D], F16, tag="xsq")
                    ssq = sspool.tile([128, 1], F32, tag="ssq")
                    if on_dve:
                        nc.vector.scalar_tensor_tensor(
                            out=xsq, in0=xs[s], scalar=INV_DM1, in1=xs[s],
                            op0=ALU.mult, op1=ALU.mult, accum_out=ssq,
                        )
                    else:
                        nc.scalar.activation(
                            xsq, xs[s], AF.Square, scale=SQ_SCALE,
                            accum_out=ssq,
                        )
                    ssqs[s] = ssq

                # ---- transpose x + stream against the w1t stationary ----
                py0 = py0_pool.tile([M, IBLK], F32)
                prevs = []
                for c2 in range(NPAIR):
                    pxt = pxt_pool.tile([128, 2 * IBLK], F16)
                    for q in range(2):
                        c = 2 * c2 + q
                        for s in range(NSUB):
                            nc.tensor.transpose(
                                pxt[:, q * IBLK + s * 128:q * IBLK + (s + 1) * 128],
                                xs[s][:, c * 128:(c + 1) * 128],
                                idh_sb,
                            )
                    xts = xtpool.tile([128, 2 * IBLK], F16, tag="xt")
                    if c2 < DVE_COPIES:
                        nc.vector.tensor_copy(xts, pxt)
                    else:
                        nc.scalar.copy(xts, pxt)
                    prevs.append((c2, xts))
                    if len(prevs) > GEMM_LAG:
                        pc2, pxts = prevs.pop(0)
                        for q in range(2):
                            c = 2 * pc2 + q
                            nc.tensor.matmul(
                                py0, w1t_sb[:, c, :],
                                pxts[:, q * IBLK:(q + 1) * IBLK],
                                start=(c == 0), stop=False,
                            )

                    # interleaved stats (keep early DVE copies early)
                    if c2 in (2, 4) and n_dve_stat < DVE_STATS:
                        emit_stat(n_dve_stat + n_act_stat, True)
                        n_dve_stat += 1
                    if c2 in (6, 8) and n_act_stat < NSUB - DVE_STATS:
                        emit_stat(n_dve_stat + n_act_stat, False)
                        n_act_stat += 1

                    # interleaved post stage for the previous block
                    if post is not None:
                        if c2 == 1:
                            t1 = post_negs(post)
                        elif c2 == 3:
                            p2 = post_l1(post, t1)
                        elif c2 == 5:
                            p3 = post_l2(post, p2)
                        elif c2 == 7:
                            y3 = post_l3(post, p3)
                        elif c2 == 9:
                            post_out(post, y3)

                while n_dve_stat < DVE_STATS:
                    emit_stat(n_dve_stat + n_act_stat, True)
                    n_dve_stat += 1
                while n_act_stat < NSUB - DVE_STATS:
                    emit_stat(n_dve_stat + n_act_stat, False)
                    n_act_stat += 1

                for pc2, pxts in prevs:
                    for q in range(2):
                        c = 2 * pc2 + q
                        nc.tensor.matmul(
                            py0, w1t_sb[:, c, :],
                            pxts[:, q * IBLK:(q + 1) * IBLK],
                            start=(c == 0), stop=(c == NCHUNK - 1),
                        )

                posts.append(post_head((b, py0, ssqs, r0)))

            # drain the remaining post stages
            while posts:
                post = posts.popleft()
                t1 = post_negs(post)
                p2 = post_l1(post, t1)
                p3 = post_l2(post, p2)
                y3 = post_l3(post, p3)
                post_out(post, y3)

    nc.compile()
    return nc


def _prep_inputs(x, w1, b1, w2, b2, w3, b3):
    x = np.ascontiguousarray(np.asarray(x, dtype=np.float32))
    w1 = np.asarray(w1, dtype=np.float32)
    w2 = np.asarray(w2, dtype=np.float32)
    w3 = np.asarray(w3, dtype=np.float32)
    b1 = np.asarray(b1, dtype=np.float32)
    b2 = np.asarray(b2, dtype=np.float32)
    b3 = np.asarray(b3, dtype=np.float32)

    # augmented stationary: cols 0..31 = w1^T, col 32 = 1/D (mean)
    w1a = np.zeros((D, M), dtype=np.float32)
    w1a[:, 0:H] = w1.T
    w1a[:, MP] = 1.0 / D
    negsm = np.zeros((M, M), dtype=np.float32)
    negsm[MP, 0:H] = -w1.astype(np.float64).sum(axis=1)
    e32 = np.zeros((M, 1), dtype=np.float32)
    e32[MP, 0] = 1.0

    common = {
        # [128, 24*33]: partition p holds w1a[c*128+p, :] for each chunk c
        "w1t": np.ascontiguousarray(
            w1a.reshape(NCHUNK, 128, M).transpose(1, 0, 2).reshape(128, NCHUNK * M)
        ).astype(np.float16),
        "w2t": np.ascontiguousarray(w2.T).astype(np.float16),
        "w3t": np.ascontiguousarray(w3.T).astype(np.float16),
        "negsm": np.ascontiguousarray(negsm).astype(np.float16),
        "e32": np.ascontiguousarray(e32).astype(np.float16),
        "b1c": np.ascontiguousarray(b1[:, None]),
        "b2c": np.ascontiguousarray(b2[:, None]),
        "b3c": np.ascontiguousarray(b3[:, None]),
        "idh": np.eye(128, dtype=np.float16),
        "idf": np.eye(128, dtype=np.float32),
        "zrowf": np.zeros((H, IBLK), dtype=np.float32),
    }
    in_maps = []
    for c in range(N_CORES):
        m = dict(common)
        m["x"] = x[c * B_CORE:(c + 1) * B_CORE]
        in_maps.append(m)
    return in_maps


def kernel(x, w1, b1, w2, b2, w3, b3):
    global LAST_EXEC_NS
    if "nc" not in _CACHE:
        _CACHE["nc"] = _build()
    nc = _CACHE["nc"]
    in_maps = _prep_inputs(x, w1, b1, w2, b2, w3, b3)
    trace = bool(int(os.environ.get("KERNEL_PROFILE", "0")))
    res = run_bass_kernel_spmd(nc, in_maps, core_ids=list(range(N_CORES)),
                               trace=trace)
    LAST_EXEC_NS = res.exec_time_ns
    out = np.concatenate([r["y"] for r in res.results], axis=0)
    return out.astype(np.float32)


# revision 27
# speedup vs baseline: 1.0909x; 1.0006x over previous
"""Trainium2 Bass kernel for nn_NeuralNet_19250043421419.

Row-normalize x (mean/std over D=3072, ddof=1) then a 3-layer MLP
(3072->32->32->10) with LeakyReLU(0.01) after every layer.

Strategy: pure data parallel over 8 NeuronCores (batch 32768 -> 4096/core).
Per core, per 512-row block:
  - DMA x in natural layout, casting fp32->fp16 in the SWDGE DMA.  GpSimd
    does nothing else, so DMA issue is never gated by compute.
  - Per-row sum(x^2)/(D-1) split between DVE (scalar_tensor_tensor with
    accum) and ACT (Square activation with accum), tunable.
  - PE transposes x into [d, i] tiles with the dedicated transpose datapath
    (fp16 PSUM output: halves both PSUM banks and copy cost); the
    PSUM->SBUF copies are split between DVE (fp16 2x rate) and ACT.
  - PE streams the transposed tiles against [w1^T | 1/D] (M=33),
    accumulating y0_raw in PSUM rows 0..31 and the row-mean in row 32.
  - Normalization folds in afterwards: (x-m)/s @ w1^T =
    (y0_raw - m * rowsum(w1)) / s.  The mean correction is a one-hot K=33
    matmul (stationary rows 0..31 zero) accumulated onto the closed PSUM
    group; a second one-hot matmul lands the mean on partition 0 where ACT
    squares it straight out of PSUM, and the 1/s scaling is a DVE multiply
    against a stream_shuffle-broadcast row, with
    var = ssq/(D-1) - m^2 * D/(D-1) (Bessel).
  - The post stage for block b-1 is interleaved into block b's emission so
    every engine sees its dependencies ready on arrival (PE post matmuls
    are spread between transpose pairs).
  - PE transposes the [10, 512] result back to natural [512, 10] and DMAs out.
"""
import os
import sys

for _p in ("/opt/trn_rl_repo", "/root/.axon_site/_ro/trn_rl_repo"):
    if os.path.isdir(_p) and _p not in sys.path:
        sys.path.append(_p)

import numpy as np

import concourse.bass as bass
import concourse.bacc as bacc
import concourse.tile as tile
from concourse import mybir
from concourse.bass_utils import run_bass_kernel_spmd

F32 = mybir.dt.float32
F16 = mybir.dt.float16
AF = mybir.ActivationFunctionType
ALU = mybir.AluOpType

N_CORES = 8
B = 32768
D = 3072
H = 32
O = 10
M = 33                     # GEMM stationary cols: [w1^T | 1/D]
MP = 32                    # partition of the mean row
B_CORE = B // N_CORES      # 4096
IBLK = 512                 # rows per block
NSUB = IBLK // 128         # 4 sub-tiles of 128 rows
NBLK = B_CORE // IBLK      # 8
NCHUNK = D // 128          # 24 contraction chunks
NPAIR = NCHUNK // 2        # 12 transpose pairs per block
DDOF = float(D) / float(D - 1)
INV_DM1 = 1.0 / float(D - 1)
SQ_SCALE = float(np.sqrt(1.0 / (D - 1)))

XBUFS = int(os.environ.get("K_XBUFS", "22"))
POST_DEPTH = int(os.environ.get("K_POST_DEPTH", "1"))
DVE_COPIES = int(os.environ.get("K_DVE_COPIES", "9"))   # of 12 per block
DVE_STATS = int(os.environ.get("K_DVE_STATS", "2"))     # of 4 per block
GEMM_LAG = int(os.environ.get("K_GEMM_LAG", "1"))       # pairs of copy lead

LAST_EXEC_NS = None
_CACHE = {}


def _build():
    nc = bacc.Bacc("TRN2", target_bir_lowering=False, debug=False, num_devices=1)

    x_d = nc.dram_tensor("x", [B_CORE, D], F32, kind="ExternalInput").ap()
    w1t_d = nc.dram_tensor("w1t", [128, NCHUNK * M], F16, kind="ExternalInput").ap()
    w2t_d = nc.dram_tensor("w2t", [H, H], F16, kind="ExternalInput").ap()
    w3t_d = nc.dram_tensor("w3t", [H, O], F16, kind="ExternalInput").ap()
    negsm_d = nc.dram_tensor("negsm", [M, M], F16, kind="ExternalInput").ap()
    e32_d = nc.dram_tensor("e32", [M, 1], F16, kind="ExternalInput").ap()
    b1_d = nc.dram_tensor("b1c", [H, 1], F32, kind="ExternalInput").ap()
    b2_d = nc.dram_tensor("b2c", [H, 1], F32, kind="ExternalInput").ap()
    b3_d = nc.dram_tensor("b3c", [O, 1], F32, kind="ExternalInput").ap()
    idh_d = nc.dram_tensor("idh", [128, 128], F16, kind="ExternalInput").ap()
    idf_d = nc.dram_tensor("idf", [128, 128], F32, kind="ExternalInput").ap()
    zf_d = nc.dram_tensor("zrowf", [H, IBLK], F32, kind="ExternalInput").ap()
    y_d = nc.dram_tensor("y", [B_CORE, O], F32, kind="ExternalOutput").ap()

    with tile.TileContext(nc) as tc:
        with tc.tile_pool(name="consts", bufs=1) as consts, \
             tc.tile_pool(name="xpool", bufs=XBUFS) as xpool, \
             tc.tile_pool(name="sqpool", bufs=3) as sqpool, \
             tc.tile_pool(name="xtpool", bufs=6) as xtpool, \
             tc.tile_pool(name="spool", bufs=2) as spool, \
             tc.tile_pool(name="sspool", bufs=12) as sspool, \
             tc.tile_pool(name="opool", bufs=2) as opool, \
             tc.tile_pool(name="pxt", bufs=4, space="PSUM") as pxt_pool, \
             tc.tile_pool(name="py0", bufs=2, space="PSUM") as py0_pool, \
             tc.tile_pool(name="pl", bufs=2, space="PSUM") as pl_pool:

            # ---- constants ----
            w1t_sb = consts.tile([128, NCHUNK, M], F16)
            nc.sync.dma_start(
                out=w1t_sb, in_=w1t_d.rearrange("p (c h) -> p c h", h=M)
            )
            w2t_sb = consts.tile([H, H], F16)
            nc.sync.dma_start(out=w2t_sb, in_=w2t_d)
            w3t_sb = consts.tile([H, O], F16)
            nc.sync.dma_start(out=w3t_sb, in_=w3t_d)
            negsm_sb = consts.tile([M, M], F16)
            nc.sync.dma_start(out=negsm_sb, in_=negsm_d)
            e32_sb = consts.tile([M, 1], F16)
            nc.sync.dma_start(out=e32_sb, in_=e32_d)
            b1_sb = consts.tile([H, 1], F32)
            nc.sync.dma_start(out=b1_sb, in_=b1_d)
            b2_sb = consts.tile([H, 1], F32)
            nc.sync.dma_start(out=b2_sb, in_=b2_d)
            b3_sb = consts.tile([O, 1], F32)
            nc.sync.dma_start(out=b3_sb, in_=b3_d)
            idh_sb = consts.tile([128, 128], F16)
            nc.sync.dma_start(out=idh_sb, in_=idh_d)
            idf_sb = consts.tile([128, 128], F32)
            nc.sync.dma_start(out=idf_sb, in_=idf_d)
            inv32 = consts.tile([H, IBLK], F32)
            nc.sync.dma_start(out=inv32, in_=zf_d)

            def post_head(st):
                """Pre-loop part of the post stage (head-start emission)."""
                _b, py0, ssqs, _r0 = st
                # ssq columns -> one [1, IBLK] psum row  [PE]
                psr = pl_pool.tile([1, IBLK], F32, tag="pl")
                for s in range(NSUB):
                    nc.tensor.matmul(
                        psr[:, s * 128:(s + 1) * 128], ssqs[s], idf_sb,
                        start=True, stop=True,
                    )
                # full psum tile -> sbuf fp16; the one-hot stationaries
                # only read row MP, rows 0..31 are finite y0 values
                mrow33 = spool.tile([M, IBLK], F16, tag="mrow")
                nc.scalar.copy(mrow33, py0)
                return st + (psr, mrow33)

            def post_negs(st):
                _b, py0, _s, _r0, psr, mrow33 = st
                # K=33 one-hot correction: py0 += negsM.T @ mrow33
                nc.tensor.matmul(py0, negsm_sb, mrow33,
                                 start=False, stop=True, skip_group_check=True)
                # mean to partition 0 via one-hot matmul, then m^2 from PSUM
                m0p = pl_pool.tile([1, IBLK], F32, tag="pl")
                nc.tensor.matmul(m0p, e32_sb, mrow33, start=True, stop=True)
                msq = spool.tile([1, IBLK], F32, tag="msq")
                nc.scalar.activation(msq, m0p, AF.Square, scale=1.0)
                # var = ssq/(D-1) - m^2 * D/(D-1);  inv = 1/sqrt(var)
                var_row = spool.tile([1, IBLK], F32, tag="vrow")
                nc.vector.scalar_tensor_tensor(
                    out=var_row, in0=msq, scalar=-DDOF,
                    in1=psr, op0=ALU.mult, op1=ALU.add,
                )
                nc.scalar.activation(inv32[0:1, :], var_row,
                                     AF.Abs_reciprocal_sqrt, scale=1.0)
                inv_b = spool.tile([H, IBLK], F32, tag="invb")
                nc.vector.stream_shuffle(inv_b, inv32, [0] * 32)
                t1 = spool.tile([H, IBLK], F32, tag="t1")
                nc.vector.tensor_mul(t1, py0[0:H, :], inv_b)
                return t1

            def post_l1(st, t1):
                h1 = spool.tile([H, IBLK], F16, tag="h1")
                nc.scalar.activation(h1, t1, AF.Prelu, bias=b1_sb, scale=1.0,
                                     alpha=0.01)
                p2 = pl_pool.tile([H, IBLK], F32, tag="pl")
                nc.tensor.matmul(p2, w2t_sb, h1, start=True, stop=True)
                return p2

            def post_l2(st, p2):
                h2 = spool.tile([H, IBLK], F16, tag="h2")
                nc.scalar.activation(h2, p2, AF.Prelu, bias=b2_sb, scale=1.0,
                                     alpha=0.01)
                p3 = pl_pool.tile([O, IBLK], F32, tag="pl")
                nc.tensor.matmul(p3, w3t_sb, h2, start=True, stop=True)
                return p3

            def post_l3(st, p3):
                y3 = spool.tile([O, IBLK], F32, tag="y3")
                nc.scalar.activation(y3, p3, AF.Prelu, bias=b3_sb, scale=1.0,
                                     alpha=0.01)
                return y3

            def post_out(st, y3):
                _b, _py0, _s, r0 = st[:4]
                pout = pl_pool.tile([128, NSUB, O], F32, tag="pl")
                for s in range(NSUB):
                    nc.tensor.transpose(
                        pout[:, s, :],
                        y3[:, s * 128:(s + 1) * 128],
                        idf_sb[0:O, 0:O],
                    )
                out_sb = opool.tile([128, NSUB, O], F32, tag="out")
                nc.vector.tensor_copy(out_sb, pout)
                nc.sync.dma_start(
                    out=y_d[r0:r0 + IBLK, :].rearrange("(s p) c -> p s c", p=128),
                    in_=out_sb,
                )

            from collections import deque
            posts = deque()
            for b in range(NBLK):
                r0 = b * IBLK
                # ---- load x block (fp32 -> fp16 cast in DMA) ----
                xs = []
                for s in range(NSUB):
                    xt = xpool.tile([128, D], F16, tag="xnat")
                    nc.gpsimd.dma_start(
                        out=xt, in_=x_d[r0 + s * 128:r0 + (s + 1) * 128, :]
                    )
                    xs.append(xt)

                post = posts.popleft() if len(posts) >= POST_DEPTH else None
                t1 = p2 = p3 = y3 = None

                # ---- per-block state ----
                ssqs = [None] * NSUB
                n_dve_stat = 0
                n_act_stat = 0

                def emit_stat(s, on_dve):
                    xsq = sqpool.tile([128, D], F16, tag="xsq")
                    ssq = sspool.tile([128, 1], F32, tag="ssq")
                    if on_dve:
                        nc.vector.scalar_tensor_tensor(
                            out=xsq, in0=xs[s], scalar=INV_DM1, in1=xs[s],
                            op0=ALU.mult, op1=ALU.mult, accum_out=ssq,
                        )
                    else:
                        nc.scalar.activation(
                            xsq, xs[s], AF.Square, scale=SQ_SCALE,
                            accum_out=ssq,
                        )
                    ssqs[s] = ssq

                # ---- transpose x + stream against the w1t stationary ----
                py0 = py0_pool.tile([M, IBLK], F32)
                prevs = []
                for c2 in range(NPAIR):
                    pxt = pxt_pool.tile([128, 2 * IBLK], F16)
                    for q in range(2):
                        c = 2 * c2 + q
                        for s in range(NSUB):
                            nc.tensor.transpose(
                                pxt[:, q * IBLK + s * 128:q * IBLK + (s + 1) * 128],
                                xs[s][:, c * 128:(c + 1) * 128],
                                idh_sb,
                            )
                    xts = xtpool.tile([128, 2 * IBLK], F16, tag="xt")
                    if c2 < DVE_COPIES:
                        nc.vector.tensor_copy(xts, pxt)
                    else:
                        nc.scalar.copy(xts, pxt)
                    prevs.append((c2, xts))
                    if len(prevs) > GEMM_LAG:
                        pc2, pxts = prevs.pop(0)
                        for q in range(2):
                            c = 2 * pc2 + q
                            nc.tensor.matmul(
                                py0, w1t_sb[:, c, :],
                                pxts[:, q * IBLK:(q + 1) * IBLK],
                                start=(c == 0), stop=False,
                            )

                    # interleaved stats (keep early DVE copies early)
                    if c2 in (2, 4) and n_dve_stat < DVE_STATS:
                        emit_stat(n_dve_stat + n_act_stat, True)
                        n_dve_stat += 1
                    if c2 in (6, 8) and n_act_stat < NSUB - DVE_STATS:
                        emit_stat(n_dve_stat + n_act_stat, False)
                        n_act_stat += 1

                    # interleaved post stage for the previous block
                    if post is not None:
                        if c2 == 1:
                            t1 = post_negs(post)
                        elif c2 == 3:
                            p2 = post_l1(post, t1)
                        elif c2 == 5:
                            p3 = post_l2(post, p2)
                        elif c2 == 7:
                            y3 = post_l3(post, p3)
                        elif c2 == 9:
                            post_out(post, y3)

                while n_dve_stat < DVE_STATS:
                    emit_stat(n_dve_stat + n_act_stat, True)
                    n_dve_stat += 1
                while n_act_stat < NSUB - DVE_STATS:
                    emit_stat(n_dve_stat + n_act_stat, False)
                    n_act_stat += 1

                for pc2, pxts in prevs:
                    for q in range(2):
                        c = 2 * pc2 + q
                        nc.tensor.matmul(
                            py0, w1t_sb[:, c, :],
                            pxts[:, q * IBLK:(q + 1) * IBLK],
                            start=(c == 0), stop=(c == NCHUNK - 1),
                        )

                posts.append(post_head((b, py0, ssqs, r0)))

            # drain the remaining post stages
            while posts:
                post = posts.popleft()
                t1 = post_negs(post)
                p2 = post_l1(post, t1)
                p3 = post_l2(post, p2)
                y3 = post_l3(post, p3)
                post_out(post, y3)

    nc.compile()
    return nc


def _prep_inputs(x, w1, b1, w2, b2, w3, b3):
    x = np.ascontiguousarray(np.asarray(x, dtype=np.float32))
    w1 = np.asarray(w1, dtype=np.float32)
    w2 = np.asarray(w2, dtype=np.float32)
    w3 = np.asarray(w3, dtype=np.float32)
    b1 = np.asarray(b1, dtype=np.float32)
    b2 = np.asarray(b2, dtype=np.float32)
    b3 = np.asarray(b3, dtype=np.float32)

    # augmented stationary: cols 0..31 = w1^T, col 32 = 1/D (mean)
    w1a = np.zeros((D, M), dtype=np.float32)
    w1a[:, 0:H] = w1.T
    w1a[:, MP] = 1.0 / D
    negsm = np.zeros((M, M), dtype=np.float32)
    negsm[MP, 0:H] = -w1.astype(np.float64).sum(axis=1)
    e32 = np.zeros((M, 1), dtype=np.float32)
    e32[MP, 0] = 1.0

    common = {
        # [128, 24*33]: partition p holds w1a[c*128+p, :] for each chunk c
        "w1t": np.ascontiguousarray(
            w1a.reshape(NCHUNK, 128, M).transpose(1, 0, 2).reshape(128, NCHUNK * M)
        ).astype(np.float16),
        "w2t": np.ascontiguousarray(w2.T).astype(np.float16),
        "w3t": np.ascontiguousarray(w3.T).astype(np.float16),
        "negsm": np.ascontiguousarray(negsm).astype(np.float16),
        "e32": np.ascontiguousarray(e32).astype(np.float16),
        "b1c": np.ascontiguousarray(b1[:, None]),
        "b2c": np.ascontiguousarray(b2[:, None]),
        "b3c": np.ascontiguousarray(b3[:, None]),
        "idh": np.eye(128, dtype=np.float16),
        "idf": np.eye(128, dtype=np.float32),
        "zrowf": np.zeros((H, IBLK), dtype=np.float32),
    }
    in_maps = []
    for c in range(N_CORES):
        m = dict(common)
        m["x"] = x[c * B_CORE:(c + 1) * B_CORE]
        in_maps.append(m)
    return in_maps


def kernel(x, w1, b1, w2, b2, w3, b3):
    global LAST_EXEC_NS
    if "nc" not in _CACHE:
        _CACHE["nc"] = _build()
    nc = _CACHE["nc"]
    in_maps = _prep_inputs(x, w1, b1, w2, b2, w3, b3)
    trace = bool(int(os.environ.get("KERNEL_PROFILE", "0")))
    res = run_bass_kernel_spmd(nc, in_maps, core_ids=list(range(N_CORES)),
                               trace=trace)
    LAST_EXEC_NS = res.exec_time_ns
    out = np.concatenate([r["y"] for r in res.results], axis=0)
    return out.astype(np.float32)


# revision 28
# speedup vs baseline: 1.1306x; 1.0364x over previous
"""Trainium2 Bass kernel for nn_NeuralNet_19250043421419.

Row-normalize x (mean/std over D=3072, ddof=1) then a 3-layer MLP
(3072->32->32->10) with LeakyReLU(0.01) after every layer.

Strategy: pure data parallel over 8 NeuronCores (batch 32768 -> 4096/core).
Per core, per 512-row block:
  - DMA x in natural layout, casting fp32->fp16 in the SWDGE DMA.  GpSimd
    does nothing else, so DMA issue is never gated by compute.
  - Per-row sum(x^2)/(D-1) split between DVE (scalar_tensor_tensor with
    accum) and ACT (Square activation with accum), tunable.
  - PE transposes x into [d, i] tiles with the dedicated transpose datapath
    (fp16 PSUM output: halves both PSUM banks and copy cost); the
    PSUM->SBUF copies are split between DVE (fp16 2x rate) and ACT.
  - PE streams the transposed tiles against [w1^T | 1/D] (M=33),
    accumulating y0_raw in PSUM rows 0..31 and the row-mean in row 32.
  - Normalization folds in afterwards: (x-m)/s @ w1^T =
    (y0_raw - m * rowsum(w1)) / s.  The mean correction is a one-hot K=33
    matmul (stationary rows 0..31 zero) accumulated onto the closed PSUM
    group; a second one-hot matmul lands the mean on partition 0 where ACT
    squares it straight out of PSUM, and the 1/s scaling is a DVE multiply
    against a stream_shuffle-broadcast row, with
    var = ssq/(D-1) - m^2 * D/(D-1) (Bessel).
  - The post stage for block b-1 is interleaved into block b's emission so
    every engine sees its dependencies ready on arrival (PE post matmuls
    are spread between transpose pairs).
  - PE transposes the [10, 512] result back to natural [512, 10] and DMAs out.
"""
import os
import sys

for _p in ("/opt/trn_rl_repo", "/root/.axon_site/_ro/trn_rl_repo"):
    if os.path.isdir(_p) and _p not in sys.path:
        sys.path.append(_p)

import numpy as np

import concourse.bass as bass
import concourse.bacc as bacc
import concourse.tile as tile
from concourse import mybir
from concourse.bass_utils import run_bass_kernel_spmd

F32 = mybir.dt.float32
F16 = mybir.dt.float16
AF = mybir.ActivationFunctionType
ALU = mybir.AluOpType

N_CORES = 8
B = 32768
D = 3072
H = 32
O = 10
M = 33                     # GEMM stationary cols: [w1^T | 1/D]
MP = 32                    # partition of the mean row
B_CORE = B // N_CORES      # 4096
IBLK = 512                 # rows per block
NSUB = IBLK // 128         # 4 sub-tiles of 128 rows
NBLK = B_CORE // IBLK      # 8
NCHUNK = D // 128          # 24 contraction chunks
NPAIR = NCHUNK // 2        # 12 transpose pairs per block
DDOF = float(D) / float(D - 1)
INV_DM1 = 1.0 / float(D - 1)
SQ_SCALE = float(np.sqrt(1.0 / (D - 1)))

XBUFS = int(os.environ.get("K_XBUFS", "22"))
POST_DEPTH = int(os.environ.get("K_POST_DEPTH", "1"))
DVE_COPIES = int(os.environ.get("K_DVE_COPIES", "9"))   # of 12 per block
DVE_STATS = int(os.environ.get("K_DVE_STATS", "2"))     # of 4 per block
GEMM_LAG = int(os.environ.get("K_GEMM_LAG", "1"))       # pairs of copy lead

LAST_EXEC_NS = None
_CACHE = {}


def _build():
    nc = bacc.Bacc("TRN2", target_bir_lowering=False, debug=False, num_devices=1)

    x_d = nc.dram_tensor("x", [B_CORE, D], F32, kind="ExternalInput").ap()
    w1t_d = nc.dram_tensor("w1t", [128, NCHUNK * M], F16, kind="ExternalInput").ap()
    w2t_d = nc.dram_tensor("w2t", [H, H], F16, kind="ExternalInput").ap()
    w3t_d = nc.dram_tensor("w3t", [H, O], F16, kind="ExternalInput").ap()
    negsm_d = nc.dram_tensor("negsm", [M, M], F16, kind="ExternalInput").ap()
    e32_d = nc.dram_tensor("e32", [M, 1], F16, kind="ExternalInput").ap()
    b1_d = nc.dram_tensor("b1c", [H, 1], F32, kind="ExternalInput").ap()
    b2_d = nc.dram_tensor("b2c", [H, 1], F32, kind="ExternalInput").ap()
    b3_d = nc.dram_tensor("b3c", [O, 1], F32, kind="ExternalInput").ap()
    idh_d = nc.dram_tensor("idh", [128, 128], F16, kind="ExternalInput").ap()
    idf_d = nc.dram_tensor("idf", [128, 128], F32, kind="ExternalInput").ap()
    zf_d = nc.dram_tensor("zrowf", [H, IBLK], F32, kind="ExternalInput").ap()
    y_d = nc.dram_tensor("y", [O, B_CORE], F32, kind="ExternalOutput").ap()

    with tile.TileContext(nc) as tc:
        with tc.tile_pool(name="consts", bufs=1) as consts, \
             tc.tile_pool(name="xpool", bufs=XBUFS) as xpool, \
             tc.tile_pool(name="sqpool", bufs=3) as sqpool, \
             tc.tile_pool(name="xtpool", bufs=6) as xtpool, \
             tc.tile_pool(name="spool", bufs=2) as spool, \
             tc.tile_pool(name="sspool", bufs=12) as sspool, \
             tc.tile_pool(name="opool", bufs=2) as opool, \
             tc.tile_pool(name="pxt", bufs=4, space="PSUM") as pxt_pool, \
             tc.tile_pool(name="py0", bufs=2, space="PSUM") as py0_pool, \
             tc.tile_pool(name="pl", bufs=2, space="PSUM") as pl_pool:

            # ---- constants ----
            w1t_sb = consts.tile([128, NCHUNK, M], F16)
            nc.sync.dma_start(
                out=w1t_sb, in_=w1t_d.rearrange("p (c h) -> p c h", h=M)
            )
            w2t_sb = consts.tile([H, H], F16)
            nc.sync.dma_start(out=w2t_sb, in_=w2t_d)
            w3t_sb = consts.tile([H, O], F16)
            nc.sync.dma_start(out=w3t_sb, in_=w3t_d)
            negsm_sb = consts.tile([M, M], F16)
            nc.sync.dma_start(out=negsm_sb, in_=negsm_d)
            e32_sb = consts.tile([M, 1], F16)
            nc.sync.dma_start(out=e32_sb, in_=e32_d)
            b1_sb = consts.tile([H, 1], F32)
            nc.sync.dma_start(out=b1_sb, in_=b1_d)
            b2_sb = consts.tile([H, 1], F32)
            nc.sync.dma_start(out=b2_sb, in_=b2_d)
            b3_sb = consts.tile([O, 1], F32)
            nc.sync.dma_start(out=b3_sb, in_=b3_d)
            idh_sb = consts.tile([128, 128], F16)
            nc.sync.dma_start(out=idh_sb, in_=idh_d)
            idf_sb = consts.tile([128, 128], F32)
            nc.sync.dma_start(out=idf_sb, in_=idf_d)
            inv32 = consts.tile([H, IBLK], F32)
            nc.sync.dma_start(out=inv32, in_=zf_d)

            def post_head(st):
                """Pre-loop part of the post stage (head-start emission)."""
                _b, py0, ssqs, _r0 = st
                # ssq columns -> one [1, IBLK] psum row  [PE]
                psr = pl_pool.tile([1, IBLK], F32, tag="pl")
                for s in range(NSUB):
                    nc.tensor.matmul(
                        psr[:, s * 128:(s + 1) * 128], ssqs[s], idf_sb,
                        start=True, stop=True,
                    )
                # full psum tile -> sbuf fp16; the one-hot stationaries
                # only read row MP, rows 0..31 are finite y0 values
                mrow33 = spool.tile([M, IBLK], F16, tag="mrow")
                nc.scalar.copy(mrow33, py0)
                return st + (psr, mrow33)

            def post_negs(st):
                _b, py0, _s, _r0, psr, mrow33 = st
                # K=33 one-hot correction: py0 += negsM.T @ mrow33
                nc.tensor.matmul(py0, negsm_sb, mrow33,
                                 start=False, stop=True, skip_group_check=True)
                # mean to partition 0 via one-hot matmul, then m^2 from PSUM
                m0p = pl_pool.tile([1, IBLK], F32, tag="pl")
                nc.tensor.matmul(m0p, e32_sb, mrow33, start=True, stop=True)
                msq = spool.tile([1, IBLK], F32, tag="msq")
                nc.scalar.activation(msq, m0p, AF.Square, scale=1.0)
                # var = ssq/(D-1) - m^2 * D/(D-1);  inv = 1/sqrt(var)
                var_row = spool.tile([1, IBLK], F32, tag="vrow")
                nc.vector.scalar_tensor_tensor(
                    out=var_row, in0=msq, scalar=-DDOF,
                    in1=psr, op0=ALU.mult, op1=ALU.add,
                )
                nc.scalar.activation(inv32[0:1, :], var_row,
                                     AF.Abs_reciprocal_sqrt, scale=1.0)
                inv_b = spool.tile([H, IBLK], F32, tag="invb")
                nc.vector.stream_shuffle(inv_b, inv32, [0] * 32)
                t1 = spool.tile([H, IBLK], F32, tag="t1")
                nc.vector.tensor_mul(t1, py0[0:H, :], inv_b)
                return t1

            def post_l1(st, t1):
                h1 = spool.tile([H, IBLK], F16, tag="h1")
                nc.scalar.activation(h1, t1, AF.Prelu, bias=b1_sb, scale=1.0,
                                     alpha=0.01)
                p2 = pl_pool.tile([H, IBLK], F32, tag="pl")
                nc.tensor.matmul(p2, w2t_sb, h1, start=True, stop=True)
                return p2

            def post_l2(st, p2):
                h2 = spool.tile([H, IBLK], F16, tag="h2")
                nc.scalar.activation(h2, p2, AF.Prelu, bias=b2_sb, scale=1.0,
                                     alpha=0.01)
                p3 = pl_pool.tile([O, IBLK], F32, tag="pl")
                nc.tensor.matmul(p3, w3t_sb, h2, start=True, stop=True)
                return p3

            def post_l3(st, p3):
                y3 = spool.tile([O, IBLK], F32, tag="y3")
                nc.scalar.activation(y3, p3, AF.Prelu, bias=b3_sb, scale=1.0,
                                     alpha=0.01)
                return y3

            def post_out(st, y3):
                _b, _py0, _s, r0 = st[:4]
                # store transposed [O, rows]; the host gather un-transposes
                nc.sync.dma_start(out=y_d[:, r0:r0 + IBLK], in_=y3)

            from collections import deque
            posts = deque()
            for b in range(NBLK):
                r0 = b * IBLK
                # ---- load x block (fp32 -> fp16 cast in DMA) ----
                xs = []
                for s in range(NSUB):
                    xt = xpool.tile([128, D], F16, tag="xnat")
                    nc.gpsimd.dma_start(
                        out=xt, in_=x_d[r0 + s * 128:r0 + (s + 1) * 128, :]
                    )
                    xs.append(xt)

                post = posts.popleft() if len(posts) >= POST_DEPTH else None
                t1 = p2 = p3 = y3 = None

                # ---- per-block state ----
                ssqs = [None] * NSUB
                n_dve_stat = 0
                n_act_stat = 0

                def emit_stat(s, on_dve):
                    xsq = sqpool.tile([128, D], F16, tag="xsq")
                    ssq = sspool.tile([128, 1], F32, tag="ssq")
                    if on_dve:
                        nc.vector.scalar_tensor_tensor(
                            out=xsq, in0=xs[s], scalar=INV_DM1, in1=xs[s],
                            op0=ALU.mult, op1=ALU.mult, accum_out=ssq,
                        )
                    else:
                        nc.scalar.activation(
                            xsq, xs[s], AF.Square, scale=SQ_SCALE,
                            accum_out=ssq,
                        )
                    ssqs[s] = ssq

                # ---- transpose x + stream against the w1t stationary ----
                py0 = py0_pool.tile([M, IBLK], F32)
                prevs = []
                for c2 in range(NPAIR):
                    pxt = pxt_pool.tile([128, 2 * IBLK], F16)
                    for q in range(2):
                        c = 2 * c2 + q
                        for s in range(NSUB):
                            nc.tensor.transpose(
                                pxt[:, q * IBLK + s * 128:q * IBLK + (s + 1) * 128],
                                xs[s][:, c * 128:(c + 1) * 128],
                                idh_sb,
                            )
                    xts = xtpool.tile([128, 2 * IBLK], F16, tag="xt")
                    if c2 < DVE_COPIES:
                        nc.vector.tensor_copy(xts, pxt)
                    else:
                        nc.scalar.copy(xts, pxt)
                    prevs.append((c2, xts))
                    if len(prevs) > GEMM_LAG:
                        pc2, pxts = prevs.pop(0)
                        for q in range(2):
                            c = 2 * pc2 + q
                            nc.tensor.matmul(
                                py0, w1t_sb[:, c, :],
                                pxts[:, q * IBLK:(q + 1) * IBLK],
                                start=(c == 0), stop=False,
                            )

                    # interleaved stats (keep early DVE copies early)
                    if c2 in (2, 4) and n_dve_stat < DVE_STATS:
                        emit_stat(n_dve_stat + n_act_stat, True)
                        n_dve_stat += 1
                    if c2 in (6, 8) and n_act_stat < NSUB - DVE_STATS:
                        emit_stat(n_dve_stat + n_act_stat, False)
                        n_act_stat += 1

                    # interleaved post stage for the previous block
                    if post is not None:
                        if c2 == 1:
                            t1 = post_negs(post)
                        elif c2 == 3:
                            p2 = post_l1(post, t1)
                        elif c2 == 5:
                            p3 = post_l2(post, p2)
                        elif c2 == 7:
                            y3 = post_l3(post, p3)
                        elif c2 == 9:
                            post_out(post, y3)

                while n_dve_stat < DVE_STATS:
                    emit_stat(n_dve_stat + n_act_stat, True)
                    n_dve_stat += 1
                while n_act_stat < NSUB - DVE_STATS:
                    emit_stat(n_dve_stat + n_act_stat, False)
                    n_act_stat += 1

                for pc2, pxts in prevs:
                    for q in range(2):
                        c = 2 * pc2 + q
                        nc.tensor.matmul(
                            py0, w1t_sb[:, c, :],
                            pxts[:, q * IBLK:(q + 1) * IBLK],
                            start=(c == 0), stop=(c == NCHUNK - 1),
                        )

                posts.append(post_head((b, py0, ssqs, r0)))

            # drain the remaining post stages
            while posts:
                post = posts.popleft()
                t1 = post_negs(post)
                p2 = post_l1(post, t1)
                p3 = post_l2(post, p2)
                y3 = post_l3(post, p3)
                post_out(post, y3)

    nc.compile()
    return nc


def _prep_inputs(x, w1, b1, w2, b2, w3, b3):
    x = np.ascontiguousarray(np.asarray(x, dtype=np.float32))
    w1 = np.asarray(w1, dtype=np.float32)
    w2 = np.asarray(w2, dtype=np.float32)
    w3 = np.asarray(w3, dtype=np.float32)
    b1 = np.asarray(b1, dtype=np.float32)
    b2 = np.asarray(b2, dtype=np.float32)
    b3 = np.asarray(b3, dtype=np.float32)

    # augmented stationary: cols 0..31 = w1^T, col 32 = 1/D (mean)
    w1a = np.zeros((D, M), dtype=np.float32)
    w1a[:, 0:H] = w1.T
    w1a[:, MP] = 1.0 / D
    negsm = np.zeros((M, M), dtype=np.float32)
    negsm[MP, 0:H] = -w1.astype(np.float64).sum(axis=1)
    e32 = np.zeros((M, 1), dtype=np.float32)
    e32[MP, 0] = 1.0

    common = {
        # [128, 24*33]: partition p holds w1a[c*128+p, :] for each chunk c
        "w1t": np.ascontiguousarray(
            w1a.reshape(NCHUNK, 128, M).transpose(1, 0, 2).reshape(128, NCHUNK * M)
        ).astype(np.float16),
        "w2t": np.ascontiguousarray(w2.T).astype(np.float16),
        "w3t": np.ascontiguousarray(w3.T).astype(np.float16),
        "negsm": np.ascontiguousarray(negsm).astype(np.float16),
        "e32": np.ascontiguousarray(e32).astype(np.float16),
        "b1c": np.ascontiguousarray(b1[:, None]),
        "b2c": np.ascontiguousarray(b2[:, None]),
        "b3c": np.ascontiguousarray(b3[:, None]),
        "idh": np.eye(128, dtype=np.float16),
        "idf": np.eye(128, dtype=np.float32),
        "zrowf": np.zeros((H, IBLK), dtype=np.float32),
    }
    in_maps = []
    for c in range(N_CORES):
        m = dict(common)
        m["x"] = x[c * B_CORE:(c + 1) * B_CORE]
        in_maps.append(m)
    return in_maps


def kernel(x, w1, b1, w2, b2, w3, b3):
    global LAST_EXEC_NS
    if "nc" not in _CACHE:
        _CACHE["nc"] = _build()
    nc = _CACHE["nc"]
    in_maps = _prep_inputs(x, w1, b1, w2, b2, w3, b3)
    trace = bool(int(os.environ.get("KERNEL_PROFILE", "0")))
    res = run_bass_kernel_spmd(nc, in_maps, core_ids=list(range(N_CORES)),
                               trace=trace)
    LAST_EXEC_NS = res.exec_time_ns
    out = np.concatenate([r["y"].T for r in res.results], axis=0)
    return np.ascontiguousarray(out, dtype=np.float32)
